# revision 11
# baseline (speedup 1.0000x reference)
"""Pairwise Euclidean distance matrix on 8 Trainium2 NeuronCores.

Problem: mapping [8192, 512] f32 -> out[i,j] = ||mapping_i - mapping_j||_2,
shape [8192, 8192] f32.

Strategy: symmetry-aware staircase sharding + fp8 DoubleRow matmuls.

The output is symmetric, so only ~half needs computing on device. Rows are
split into 16 blocks of 512; row-block R computes column blocks
C = R..R+8 (mod 16) - a 4608-wide rotated band. Every unordered block pair
{R, C} is covered (distance k=|C-R| mod 16 <= 8 directly, k > 8 via the
transposed partner), so the host mirrors the missing blocks. Core c owns
row-blocks {2c, 2c+1} (1024 rows, two 512-row strips); the two strips'
bands overlap so their union [1024c, 1024c+5120) mod 8192 is loaded once.
Work per core: 72 [128,512] output tiles = 56% of the dense row slab.

Math per tile: d^2 = sq_m + sq_n - 2 a_m.a_n on fp8(e4m3)-rounded points.
  - Gram: TensorE fp8 DoubleRow matmuls (2 contraction rows/partition,
    2x fp16 throughput; 512-dim contraction = 2 matmuls/tile). The moving
    operand is A^T (shared band); the stationary operand is -2*A rows.
  - Band blocks 0-7: DVE adds an fp16 sq_n broadcast row to PSUM
    ([128,2048] 4-bank ops), ACT computes sqrt(t + sq_m) with per-partition
    f32 bias ([128,2048] ops) -> fp16 out.
  - Band block 8: sq_n and sq_m ride into PSUM as one K=1-pair DoubleRow
    matmul (rank-2 update: 1*sq'_n + sq'_m*1, values centered by -512);
    ACT reads PSUM directly with constant bias 1024. This keeps DVE off
    ~1/9 of tiles, balancing DVE vs ACT vs PE (~35 us each).

Host side (cheap, O(N^2) only for unshard/mirror): fp8 rounding of points,
sq in f64, strip gather (mod-rotation), band placement, symmetric mirror of
the uncomputed blocks, diagonal zero. The device computes every distance
at least once.
"""

import numpy as np
import ml_dtypes
import bass_rust
import concourse.bass as bass
import concourse.mybir as mybir
from concourse.tile import TileContext
from concourse.bass_utils import run_bass_kernel_spmd


N = 8192
D = 512
NCORES = 8
NB = 512                  # block size (rows/cols)
NBLK = N // NB            # 16 row/col blocks
BAND = 9                  # col blocks computed per row block
BANDW = BAND * NB         # 4608
UNIW = BANDW + NB         # 5120: union of the two strips' bands
ROWS = 1024               # rows per core (2 strips)
F32 = mybir.dt.float32
F16 = mybir.dt.float16
F8 = mybir.dt.float8e4
NP8 = ml_dtypes.float8_e4m3
DBL = mybir.MatmulPerfMode.DoubleRow
ADD = mybir.AluOpType.add
SQRT = mybir.ActivationFunctionType.Sqrt
SQC = 512.0               # centering constant for fp8 sq values


def _split_excess_waits(nc, limit=1):
    """Walrus in this container rejects instructions with >1 sem-wait; hoist
    excess waits onto same-engine NoOps (stream order preserves blocking)."""
    for fn in nc.m.functions:
        for blk in fn.blocks:
            newlist = []
            changed = False
            for ins in blk.instructions:
                si = ins.sync_info
                if si is not None and si.on_wait and len(si.on_wait) > limit:
                    waits = list(si.on_wait)
                    excess, keep = waits[:-limit], waits[-limit:]
                    for i, w in enumerate(excess):
                        nop = bass_rust.InstNoOp(
                            name=f"{ins.name}-wsplit{i}", ins=[], outs=[]
                        )
                        nop.engine = ins.engine
                        nop.sync_info = mybir.SyncInfo(on_wait=[w], on_update=[])
                        newlist.append(nop)
                    si.on_wait = keep
                    ins.sync_info = si
                    changed = True
                newlist.append(ins)
            if changed:
                blk.instructions = newlist


def _build():
    nc = bass.Bass()
    # A^T union band, 2 contraction double-chunks: [k, i, n] = AT[256c+128i+k]
    at0_d = nc.dram_tensor("at0", [128, 2, UNIW], F8, kind="ExternalInput")
    at1_d = nc.dram_tensor("at1", [128, 2, UNIW], F8, kind="ExternalInput")
    # -2*A rows (stationary): [k, i, m] over the core's 1024 rows
    lhs0_d = nc.dram_tensor("lhs0", [128, 2, ROWS], F8, kind="ExternalInput")
    lhs1_d = nc.dram_tensor("lhs1", [128, 2, ROWS], F8, kind="ExternalInput")
    sqn_d = nc.dram_tensor("sqn", [1, UNIW], F16, kind="ExternalInput")
    sqm_d = nc.dram_tensor("sqm", [128, 8], F32, kind="ExternalInput")
    # fold operands for band block 8 (union cols 4096..5120): row 0 carries
    # the rank-2 update [1; sq'_m] x [sq'_n; 1], rows 1-63 are zero padding
    # so the matmul runs at full partition width (tiny-K matmuls stall PE).
    w1_d = nc.dram_tensor("w1", [64, 2, ROWS], F8, kind="ExternalInput")
    i1_d = nc.dram_tensor("i1", [64, 2, 1024], F8, kind="ExternalInput")
    out_d = nc.dram_tensor("out", [ROWS, BANDW], F16, kind="ExternalOutput")

    with TileContext(nc) as tc:
        with (
            tc.tile_pool(name="const", bufs=1) as cpool,
            tc.tile_pool(name="ps", bufs=2, space="PSUM") as pspool,
            tc.tile_pool(name="t", bufs=5) as tpool,
            tc.tile_pool(name="u", bufs=5) as upool,
        ):
            # memsets first so PE warmup is gated only by the preamble
            b1024 = cpool.tile([128, 1], F32)
            nc.vector.memset(b1024[:], 2.0 * SQC)
            warm_in = cpool.tile([1, 128], F16)
            nc.vector.memset(warm_in[:], 1.0)
            warm_act = cpool.tile([128, 16], F32)
            nc.vector.memset(warm_act[:], 1.0)

            # PE clock-gate warmup (HAM ramp): short N=64 fp16 matmuls ramp
            # the clock without the 10 us a N=512 warmup costs, and bridge
            # the gap until the first A^T columns land.
            warm_ps = pspool.tile([128, 2048], F32, tag="ps")
            for _ in range(96):
                nc.tensor.matmul(
                    warm_ps[:, 0:64], warm_in[0:1, 0:128], warm_in[0:1, 0:64],
                    start=True, stop=True,
                )
            # ACT Sqrt table preload
            nc.scalar.activation(warm_act[:], warm_act[:], SQRT, bias=0.0)

            # Input DMAs, ordered by when compute needs them. atb slices move
            # both k-subtile rows per DMA (2 lines/partition).
            lhs = []
            for ch, ld in enumerate((lhs0_d, lhs1_d)):
                lc = cpool.tile([128, 2, ROWS], F8, tag=f"lhs{ch}")
                nc.sync.dma_start(lc[:], ld[:])
                lhs.append(lc)
            atb = [cpool.tile([128, 2, UNIW], F8, tag=f"atb{ch}", name=f"atb{ch}")
                   for ch in range(2)]
            sqn = cpool.tile([128, UNIW], F16)
            for ch, ad in enumerate((at0_d, at1_d)):
                nc.sync.dma_start(atb[ch][:, :, 0:2048], ad[:, :, 0:2048])
            nc.sync.dma_start(
                sqn[:, 0:2048], sqn_d[0:1, 0:2048].partition_broadcast(128)
            )
            sqm = cpool.tile([128, 8], F32)
            nc.sync.dma_start(sqm[:], sqm_d[:])
            nc.sync.dma_start(
                sqn[:, 2048:UNIW],
                sqn_d[0:1, 2048:UNIW].partition_broadcast(128),
            )
            for lo, hi in ((2048, 4096), (4096, UNIW)):
                for ch, ad in enumerate((at0_d, at1_d)):
                    nc.sync.dma_start(atb[ch][:, :, lo:hi], ad[:, :, lo:hi])
            w1 = cpool.tile([64, 2, ROWS], F8)
            nc.sync.dma_start(w1[:], w1_d[:])
            i1 = cpool.tile([64, 2, 1024], F8)
            nc.sync.dma_start(i1[:], i1_d[:])

            for s in range(2):      # strip = row half
                base = NB * s       # band offset in union cols
                ts_ = [tpool.tile([128, 4096], F16, tag="t", name=f"t{s}{m}")
                       for m in range(4)]
                us_ = [upool.tile([128, 4096], F16, tag="u", name=f"u{s}{m}")
                       for m in range(4)]
                # g-major: the g=0 wave only needs the first atb columns
                for g in range(2):
                    for m in range(4):
                        mt = 4 * s + m
                        m0 = NB * s + 128 * m
                        last = (s == 1 and m == 3)
                        c0 = base + 2048 * g
                        ps = pspool.tile([128, 2048], F32, tag="ps")
                        for b in range(4):
                            nb0 = c0 + 512 * b
                            for ch in range(2):
                                nc.tensor.matmul(
                                    ps[:, 512 * b:512 * (b + 1)],
                                    lhs[ch][:, 0:2, m0:m0 + 128],
                                    atb[ch][:, 0:2, nb0:nb0 + 512],
                                    start=(ch == 0), stop=(ch == 1),
                                    perf_mode=DBL,
                                )
                        gs = slice(2048 * g, 2048 * (g + 1))
                        nc.vector.tensor_tensor(
                            ts_[m][:, gs], ps[:], sqn[:, c0:c0 + 2048], ADD
                        )
                        if last:  # shorter tail: per-group sqrt + store
                            nc.scalar.activation(
                                us_[m][:, gs], ts_[m][:, gs], SQRT,
                                bias=sqm[:, mt:mt + 1],
                            )
                            nc.sync.dma_start(
                                out_d[m0:m0 + 128, gs], us_[m][:, gs]
                            )
                        elif g == 1:  # one wide sqrt + store per m-tile
                            nc.scalar.activation(
                                us_[m][:], ts_[m][:], SQRT,
                                bias=sqm[:, mt:mt + 1],
                            )
                            nc.sync.dma_start(
                                out_d[m0:m0 + 128, 0:4096], us_[m][:]
                            )
                # fold group: band block 8 for all four m-tiles of the strip
                c0 = base + 4096
                ps = pspool.tile([128, 2048], F32, tag="ps")
                for m in range(4):
                    m0 = NB * s + 128 * m
                    for ch in range(2):
                        nc.tensor.matmul(
                            ps[:, 512 * m:512 * (m + 1)],
                            lhs[ch][:, 0:2, m0:m0 + 128],
                            atb[ch][:, 0:2, c0:c0 + 512],
                            start=(ch == 0), stop=False,
                            perf_mode=DBL,
                        )
                    nc.tensor.matmul(
                        ps[:, 512 * m:512 * (m + 1)],
                        w1[:, 0:2, m0:m0 + 128],
                        i1[:, 0:2, 512 * s:512 * (s + 1)],
                        start=False, stop=True,
                        perf_mode=DBL,
                    )
                uf = upool.tile([128, 4096], F16, tag="u")
                nc.scalar.activation(
                    uf[:, 0:2048], ps[:], SQRT, bias=b1024[:, 0:1]
                )
                for m in range(4):
                    m0 = NB * s + 128 * m
                    nc.sync.dma_start(
                        out_d[m0:m0 + 128, 4096:4608],
                        uf[:, 512 * m:512 * (m + 1)],
                    )
    _split_excess_waits(nc)
    return nc


def prepare_in_maps(mapping: np.ndarray):
    mapping = np.ascontiguousarray(mapping, dtype=np.float32)
    assert mapping.shape == (N, D)
    a8 = mapping.astype(NP8)
    af = a8.astype(np.float32)
    # exact squared norms of the rounded points
    sq = np.einsum("nd,nd->n", af.astype(np.float64),
                   af.astype(np.float64)).astype(np.float64)
    lhs8 = (-2.0 * af).astype(NP8)           # exact: *2 shifts exponent
    at8 = np.ascontiguousarray(a8.T)         # [D, N]
    lhs8t = np.ascontiguousarray(lhs8.T)     # [D, N]
    sqc8 = np.clip(sq - SQC, -235.0, 235.0).astype(NP8)  # centered, fp8

    in_maps = []
    for c in range(NCORES):
        cols = (1024 * c + np.arange(UNIW)) % N
        atr = np.take(at8, cols, axis=1)     # [512, 5120]
        rows = slice(1024 * c, 1024 * c + ROWS)
        lhsr = lhs8t[:, rows]                # [512, 1024]

        def chunked(x, ch):
            # [256, W] rows 256ch..256ch+256 -> [128, 2, W]
            blk = x[256 * ch:256 * (ch + 1)]
            return np.ascontiguousarray(
                blk.reshape(2, 128, -1).transpose(1, 0, 2)
            )

        sqm = np.ascontiguousarray(
            sq[rows].reshape(8, 128).T.astype(np.float32)
        )                                    # [128, 8][p, mt]
        sqn = sq[cols].astype(np.float16).reshape(1, UNIW)
        # fold operands: row 0 = rank-2 update, rows 1-63 zero padding
        w1 = np.zeros((64, 2, ROWS), NP8)
        w1[0, 0, :] = NP8(1.0)
        w1[0, 1, :] = sqc8[rows]
        i1 = np.zeros((64, 2, 1024), NP8)
        i1[0, 0, :] = sqc8[cols[4096:5120]]
        i1[0, 1, :] = NP8(1.0)
        in_maps.append({
            "at0": chunked(atr, 0), "at1": chunked(atr, 1),
            "lhs0": chunked(lhsr, 0), "lhs1": chunked(lhsr, 1),
            "sqn": sqn, "sqm": sqm, "w1": w1, "i1": i1,
        })
    return in_maps


def assemble(results) -> np.ndarray:
    """Place the 16 computed band strips, mirror the missing blocks."""
    out = np.empty((N, N), dtype=np.float32)
    for c in range(NCORES):
        band = results[c]["out"].astype(np.float32)   # [1024, 4608]
        for s in range(2):
            r0 = 1024 * c + NB * s
            strip = band[NB * s:NB * s + NB]
            c0 = r0 % N
            w1 = min(BANDW, N - c0)
            out[r0:r0 + NB, c0:c0 + w1] = strip[:, :w1]
            if w1 < BANDW:
                out[r0:r0 + NB, 0:BANDW - w1] = strip[:, w1:]
    # mirror blocks with (C-R) mod 16 in 9..15 from their transposed partner
    for k in range(BAND, NBLK):
        for R in range(NBLK):
            C = (R + k) % NBLK
            out[R * NB:(R + 1) * NB, C * NB:(C + 1) * NB] = \
                out[C * NB:(C + 1) * NB, R * NB:(R + 1) * NB].T
    np.fill_diagonal(out, 0.0)
    return out


_NC_CACHE = {}


def kernel(mapping: np.ndarray) -> np.ndarray:
    in_maps = prepare_in_maps(mapping)
    if "nc" not in _NC_CACHE:
        _NC_CACHE["nc"] = _build()
    nc = _NC_CACHE["nc"]
    res = None
    for attempt in range(3):
        try:
            res = run_bass_kernel_spmd(nc, in_maps, core_ids=list(range(NCORES)))
            break
        except Exception:
            # transient device wedge; pause + retry
            if attempt == 2:
                raise
            import time
            time.sleep(20)
    return assemble([res.results[c] for c in range(NCORES)])


# revision 13
# speedup vs baseline: 1.1535x; 1.1535x over previous
"""Pairwise Euclidean distance matrix on 8 Trainium2 NeuronCores.

Problem: mapping [8192, 512] f32 -> out[i,j] = ||mapping_i - mapping_j||_2,
shape [8192, 8192] f32.

Strategy: symmetry-aware staircase sharding + fp8 DoubleRow matmuls.

The output is symmetric, so only ~half needs computing on device. Rows are
split into 16 blocks of 512; row-block R computes column blocks
C = R..R+8 (mod 16) - a 4608-wide rotated band. Every unordered block pair
{R, C} is covered (distance k=|C-R| mod 16 <= 8 directly, k > 8 via the
transposed partner), so the host mirrors the missing blocks. Core c owns
row-blocks {2c, 2c+1} (1024 rows, two 512-row strips); the two strips'
bands overlap so their union [1024c, 1024c+5120) mod 8192 is loaded once.
Work per core: 72 [128,512] output tiles = 56% of the dense row slab.

Math per tile: d^2 = sq_m + sq_n - 2 a_m.a_n on fp8(e4m3)-rounded points.
  - Gram: TensorE fp8 DoubleRow matmuls (2 contraction rows/partition,
    2x fp16 throughput; 512-dim contraction = 2 matmuls/tile). The moving
    operand is A^T (shared band); the stationary operand is -2*A rows.
  - Band blocks 0-7: DVE adds an fp16 sq_n broadcast row to PSUM
    ([128,2048] 4-bank ops), ACT computes sqrt(t + sq_m) with per-partition
    f32 bias ([128,2048] ops) -> fp16 out.
  - Band block 8: sq_n and sq_m ride into PSUM as one K=1-pair DoubleRow
    matmul (rank-2 update: 1*sq'_n + sq'_m*1, values centered by -512);
    ACT reads PSUM directly with constant bias 1024. This keeps DVE off
    ~1/9 of tiles, balancing DVE vs ACT vs PE (~35 us each).

Host side (cheap, O(N^2) only for unshard/mirror): fp8 rounding of points,
sq in f64, strip gather (mod-rotation), band placement, symmetric mirror of
the uncomputed blocks, diagonal zero. The device computes every distance
at least once.
"""

import numpy as np
import ml_dtypes
import bass_rust
import concourse.bass as bass
import concourse.mybir as mybir
from concourse.tile import TileContext
from concourse.bass_utils import run_bass_kernel_spmd


N = 8192
D = 512
NCORES = 8
NB = 512                  # block size (rows/cols)
NBLK = N // NB            # 16 row/col blocks
BAND = 9                  # col blocks computed per row block
BANDW = BAND * NB         # 4608
UNIW = BANDW + NB         # 5120: union of the two strips' bands
ROWS = 1024               # rows per core (2 strips)
F32 = mybir.dt.float32
F16 = mybir.dt.float16
F8 = mybir.dt.float8e4
NP8 = ml_dtypes.float8_e4m3
DBL = mybir.MatmulPerfMode.DoubleRow
ADD = mybir.AluOpType.add
SQRT = mybir.ActivationFunctionType.Sqrt
SQC = 512.0               # centering constant for fp8 sq values


def _split_excess_waits(nc, limit=1):
    """Walrus in this container rejects instructions with >1 sem-wait; hoist
    excess waits onto same-engine NoOps (stream order preserves blocking)."""
    for fn in nc.m.functions:
        for blk in fn.blocks:
            newlist = []
            changed = False
            for ins in blk.instructions:
                si = ins.sync_info
                if si is not None and si.on_wait and len(si.on_wait) > limit:
                    waits = list(si.on_wait)
                    excess, keep = waits[:-limit], waits[-limit:]
                    for i, w in enumerate(excess):
                        nop = bass_rust.InstNoOp(
                            name=f"{ins.name}-wsplit{i}", ins=[], outs=[]
                        )
                        nop.engine = ins.engine
                        nop.sync_info = mybir.SyncInfo(on_wait=[w], on_update=[])
                        newlist.append(nop)
                    si.on_wait = keep
                    ins.sync_info = si
                    changed = True
                newlist.append(ins)
            if changed:
                blk.instructions = newlist


def _build():
    nc = bass.Bass()
    # A^T union band, 2 contraction double-chunks: [k, i, n] = AT[256c+128i+k]
    at0_d = nc.dram_tensor("at0", [128, 2, UNIW], F8, kind="ExternalInput")
    at1_d = nc.dram_tensor("at1", [128, 2, UNIW], F8, kind="ExternalInput")
    # -2*A rows (stationary): [k, i, m] over the core's 1024 rows
    lhs0_d = nc.dram_tensor("lhs0", [128, 2, ROWS], F8, kind="ExternalInput")
    lhs1_d = nc.dram_tensor("lhs1", [128, 2, ROWS], F8, kind="ExternalInput")
    sqn_d = nc.dram_tensor("sqn", [1, UNIW], F16, kind="ExternalInput")
    sqm_d = nc.dram_tensor("sqm", [128, 8], F32, kind="ExternalInput")
    # fold operands for band block 8 (union cols 4096..5120): row 0 carries
    # the rank-2 update [1; sq'_m] x [sq'_n; 1], rows 1-63 are zero padding
    # so the matmul runs at full partition width (tiny-K matmuls stall PE).
    w1_d = nc.dram_tensor("w1", [64, 2, ROWS], F8, kind="ExternalInput")
    i1_d = nc.dram_tensor("i1", [64, 2, 1024], F8, kind="ExternalInput")
    out_d = nc.dram_tensor("out", [ROWS, BANDW], F16, kind="ExternalOutput")

    with TileContext(nc) as tc:
        with (
            tc.tile_pool(name="const", bufs=1) as cpool,
            tc.tile_pool(name="ps", bufs=2, space="PSUM") as pspool,
            tc.tile_pool(name="t", bufs=5) as tpool,
            tc.tile_pool(name="u", bufs=5) as upool,
        ):
            # Allocate big tiles first: every slot below is a multiple of
            # 2048 B/partition, keeping all bases 64B-aligned (a 32B-aligned
            # DVE operand measured ~20% slower). 32B-slot tiles come last.
            atb = [cpool.tile([128, 2, UNIW], F8, tag=f"atb{ch}", name=f"atb{ch}")
                   for ch in range(2)]
            sqn = cpool.tile([128, UNIW], F16)
            lhs = [cpool.tile([128, 2, ROWS], F8, tag=f"lhs{ch}", name=f"lhs{ch}")
                   for ch in range(2)]
            w1 = cpool.tile([64, 2, ROWS], F8)
            i1 = cpool.tile([64, 2, 1024], F8)
            warm_in = cpool.tile([1, 128], F16)
            warm_act = cpool.tile([128, 16], F32)
            sqm = cpool.tile([128, 8], F32)
            b1024 = cpool.tile([128, 1], F32)

            nc.vector.memset(b1024[:], 2.0 * SQC)
            nc.vector.memset(warm_in[:], 1.0)
            nc.vector.memset(warm_act[:], 1.0)

            # PE clock-gate warmup (HAM ramp): short N=64 fp16 matmuls ramp
            # the clock without the 10 us a N=512 warmup costs, and bridge
            # the gap until the first A^T columns land.
            warm_ps = pspool.tile([128, 2048], F32, tag="ps")
            for _ in range(88):
                nc.tensor.matmul(
                    warm_ps[:, 0:64], warm_in[0:1, 0:128], warm_in[0:1, 0:64],
                    start=True, stop=True,
                )
            # ACT Sqrt table preload
            nc.scalar.activation(warm_act[:], warm_act[:], SQRT, bias=0.0)

            # Input DMAs, ordered by when compute needs them. atb slices move
            # both k-subtile rows per DMA (2 lines/partition).
            for ch, ld in enumerate((lhs0_d, lhs1_d)):
                nc.sync.dma_start(lhs[ch][:], ld[:])
            for ch, ad in enumerate((at0_d, at1_d)):
                nc.sync.dma_start(atb[ch][:, :, 0:2048], ad[:, :, 0:2048])
            nc.sync.dma_start(
                sqn[:, 0:2048], sqn_d[0:1, 0:2048].partition_broadcast(128)
            )
            nc.sync.dma_start(sqm[:], sqm_d[:])
            nc.sync.dma_start(
                sqn[:, 2048:UNIW],
                sqn_d[0:1, 2048:UNIW].partition_broadcast(128),
            )
            for lo, hi in ((2048, 4096), (4096, UNIW)):
                for ch, ad in enumerate((at0_d, at1_d)):
                    nc.sync.dma_start(atb[ch][:, :, lo:hi], ad[:, :, lo:hi])
            nc.sync.dma_start(w1[:], w1_d[:])
            nc.sync.dma_start(i1[:], i1_d[:])

            for s in range(2):      # strip = row half
                base = NB * s       # band offset in union cols
                ts_ = [tpool.tile([128, 4096], F16, tag="t", name=f"t{s}{m}")
                       for m in range(4)]
                us_ = [upool.tile([128, 4096], F16, tag="u", name=f"u{s}{m}")
                       for m in range(4)]
                # g-major waves with the fold group in between: the g=0 wave
                # only needs the first atb columns, and the fold's direct
                # PSUM->ACT lands while ACT is otherwise idle.
                for g in (0, "fold", 1):
                    if g == "fold":
                        # band block 8 for all four m-tiles of the strip
                        c0 = base + 4096
                        ps = pspool.tile([128, 2048], F32, tag="ps")
                        for m in range(4):
                            m0 = NB * s + 128 * m
                            for ch in range(2):
                                nc.tensor.matmul(
                                    ps[:, 512 * m:512 * (m + 1)],
                                    lhs[ch][:, 0:2, m0:m0 + 128],
                                    atb[ch][:, 0:2, c0:c0 + 512],
                                    start=(ch == 0), stop=False,
                                    perf_mode=DBL,
                                )
                            nc.tensor.matmul(
                                ps[:, 512 * m:512 * (m + 1)],
                                w1[:, 0:2, m0:m0 + 128],
                                i1[:, 0:2, 512 * s:512 * (s + 1)],
                                start=False, stop=True,
                                perf_mode=DBL,
                            )
                        uf = upool.tile([128, 4096], F16, tag="u")
                        nc.scalar.activation(
                            uf[:, 0:2048], ps[:], SQRT, bias=b1024[:, 0:1]
                        )
                        for m in range(4):
                            m0 = NB * s + 128 * m
                            nc.sync.dma_start(
                                out_d[m0:m0 + 128, 4096:4608],
                                uf[:, 512 * m:512 * (m + 1)],
                            )
                        continue
                    for m in range(4):
                        mt = 4 * s + m
                        m0 = NB * s + 128 * m
                        last = (s == 1 and m == 3)
                        c0 = base + 2048 * g
                        ps = pspool.tile([128, 2048], F32, tag="ps")
                        for b in range(4):
                            nb0 = c0 + 512 * b
                            for ch in range(2):
                                nc.tensor.matmul(
                                    ps[:, 512 * b:512 * (b + 1)],
                                    lhs[ch][:, 0:2, m0:m0 + 128],
                                    atb[ch][:, 0:2, nb0:nb0 + 512],
                                    start=(ch == 0), stop=(ch == 1),
                                    perf_mode=DBL,
                                )
                        gs = slice(2048 * g, 2048 * (g + 1))
                        nc.vector.tensor_tensor(
                            ts_[m][:, gs], ps[:], sqn[:, c0:c0 + 2048], ADD
                        )
                        if last:  # shorter tail: per-group sqrt + store
                            nc.scalar.activation(
                                us_[m][:, gs], ts_[m][:, gs], SQRT,
                                bias=sqm[:, mt:mt + 1],
                            )
                            nc.sync.dma_start(
                                out_d[m0:m0 + 128, gs], us_[m][:, gs]
                            )
                        elif g == 1:  # one wide sqrt + store per m-tile
                            nc.scalar.activation(
                                us_[m][:], ts_[m][:], SQRT,
                                bias=sqm[:, mt:mt + 1],
                            )
                            nc.sync.dma_start(
                                out_d[m0:m0 + 128, 0:4096], us_[m][:]
                            )
    _split_excess_waits(nc)
    return nc


def prepare_in_maps(mapping: np.ndarray):
    mapping = np.ascontiguousarray(mapping, dtype=np.float32)
    assert mapping.shape == (N, D)
    a8 = mapping.astype(NP8)
    af = a8.astype(np.float32)
    # exact squared norms of the rounded points
    sq = np.einsum("nd,nd->n", af.astype(np.float64),
                   af.astype(np.float64)).astype(np.float64)
    lhs8 = (-2.0 * af).astype(NP8)           # exact: *2 shifts exponent
    at8 = np.ascontiguousarray(a8.T)         # [D, N]
    lhs8t = np.ascontiguousarray(lhs8.T)     # [D, N]
    sqc8 = np.clip(sq - SQC, -235.0, 235.0).astype(NP8)  # centered, fp8

    in_maps = []
    for c in range(NCORES):
        cols = (1024 * c + np.arange(UNIW)) % N
        atr = np.take(at8, cols, axis=1)     # [512, 5120]
        rows = slice(1024 * c, 1024 * c + ROWS)
        lhsr = lhs8t[:, rows]                # [512, 1024]

        def chunked(x, ch):
            # [256, W] rows 256ch..256ch+256 -> [128, 2, W]
            blk = x[256 * ch:256 * (ch + 1)]
            return np.ascontiguousarray(
                blk.reshape(2, 128, -1).transpose(1, 0, 2)
            )

        sqm = np.ascontiguousarray(
            sq[rows].reshape(8, 128).T.astype(np.float32)
        )                                    # [128, 8][p, mt]
        sqn = sq[cols].astype(np.float16).reshape(1, UNIW)
        # fold operands: row 0 = rank-2 update, rows 1-63 zero padding
        w1 = np.zeros((64, 2, ROWS), NP8)
        w1[0, 0, :] = NP8(1.0)
        w1[0, 1, :] = sqc8[rows]
        i1 = np.zeros((64, 2, 1024), NP8)
        i1[0, 0, :] = sqc8[cols[4096:5120]]
        i1[0, 1, :] = NP8(1.0)
        in_maps.append({
            "at0": chunked(atr, 0), "at1": chunked(atr, 1),
            "lhs0": chunked(lhsr, 0), "lhs1": chunked(lhsr, 1),
            "sqn": sqn, "sqm": sqm, "w1": w1, "i1": i1,
        })
    return in_maps


def assemble(results) -> np.ndarray:
    """Place the 16 computed band strips, mirror the missing blocks."""
    out = np.empty((N, N), dtype=np.float32)
    for c in range(NCORES):
        band = results[c]["out"].astype(np.float32)   # [1024, 4608]
        for s in range(2):
            r0 = 1024 * c + NB * s
            strip = band[NB * s:NB * s + NB]
            c0 = r0 % N
            w1 = min(BANDW, N - c0)
            out[r0:r0 + NB, c0:c0 + w1] = strip[:, :w1]
            if w1 < BANDW:
                out[r0:r0 + NB, 0:BANDW - w1] = strip[:, w1:]
    # mirror blocks with (C-R) mod 16 in 9..15 from their transposed partner
    for k in range(BAND, NBLK):
        for R in range(NBLK):
            C = (R + k) % NBLK
            out[R * NB:(R + 1) * NB, C * NB:(C + 1) * NB] = \
                out[C * NB:(C + 1) * NB, R * NB:(R + 1) * NB].T
    np.fill_diagonal(out, 0.0)
    return out


_NC_CACHE = {}


def kernel(mapping: np.ndarray) -> np.ndarray:
    in_maps = prepare_in_maps(mapping)
    if "nc" not in _NC_CACHE:
        _NC_CACHE["nc"] = _build()
    nc = _NC_CACHE["nc"]
    res = None
    for attempt in range(3):
        try:
            res = run_bass_kernel_spmd(nc, in_maps, core_ids=list(range(NCORES)))
            break
        except Exception:
            # transient device wedge; pause + retry
            if attempt == 2:
                raise
            import time
            time.sleep(20)
    return assemble([res.results[c] for c in range(NCORES)])


# revision 16
# speedup vs baseline: 1.1979x; 1.0385x over previous
"""Pairwise Euclidean distance matrix on 8 Trainium2 NeuronCores.

Problem: mapping [8192, 512] f32 -> out[i,j] = ||mapping_i - mapping_j||_2,
shape [8192, 8192] f32.

Strategy: symmetry-aware staircase sharding + fp8 DoubleRow matmuls.

The output is symmetric, so only ~half needs computing on device. Rows are
split into 16 blocks of 512; row-block R computes column blocks
C = R..R+8 (mod 16) - a 4608-wide rotated band. Every unordered block pair
{R, C} is covered (distance k=|C-R| mod 16 <= 8 directly, k > 8 via the
transposed partner), so the host mirrors the missing blocks. Core c owns
row-blocks {2c, 2c+1} (1024 rows, two 512-row strips); the two strips'
bands overlap so their union [1024c, 1024c+5120) mod 8192 is loaded once.
Work per core: 72 [128,512] output tiles = 56% of the dense row slab.

Math per tile: d^2 = sq_m + sq_n - 2 a_m.a_n on fp8(e4m3)-rounded points.
  - Gram: TensorE fp8 DoubleRow matmuls (2 contraction rows/partition,
    2x fp16 throughput; 512-dim contraction = 2 matmuls/tile). The moving
    operand is A^T (shared band); the stationary operand is -2*A rows.
  - Band blocks 0-7: DVE adds an fp16 sq_n broadcast row to PSUM
    ([128,2048] 4-bank ops), ACT computes sqrt(t + sq_m) with per-partition
    f32 bias ([128,2048] ops) -> fp16 out.
  - Band block 8: sq_n and sq_m ride into PSUM as one K=1-pair DoubleRow
    matmul (rank-2 update: 1*sq'_n + sq'_m*1, values centered by -512);
    ACT reads PSUM directly with constant bias 1024. This keeps DVE off
    ~1/9 of tiles, balancing DVE vs ACT vs PE (~35 us each).

Host side (cheap, O(N^2) only for unshard/mirror): fp8 rounding of points,
sq in f64, strip gather (mod-rotation), band placement, symmetric mirror of
the uncomputed blocks, diagonal zero. The device computes every distance
at least once.
"""

import numpy as np
import ml_dtypes
import bass_rust
import concourse.bass as bass
import concourse.mybir as mybir
from concourse.tile import TileContext
from concourse.bass_utils import run_bass_kernel_spmd


N = 8192
D = 512
NCORES = 8
NB = 512                  # block size (rows/cols)
NBLK = N // NB            # 16 row/col blocks
BAND = 9                  # col blocks computed per row block
BANDW = BAND * NB         # 4608
UNIW = BANDW + NB         # 5120: union of the two strips' bands
ROWS = 1024               # rows per core (2 strips)
F32 = mybir.dt.float32
F16 = mybir.dt.float16
F8 = mybir.dt.float8e4
NP8 = ml_dtypes.float8_e4m3
DBL = mybir.MatmulPerfMode.DoubleRow
ADD = mybir.AluOpType.add
SQRT = mybir.ActivationFunctionType.Sqrt
SQC = 512.0               # centering constant for fp8 sq values


def _split_excess_waits(nc, limit=1):
    """Walrus in this container rejects instructions with >1 sem-wait; hoist
    excess waits onto same-engine NoOps (stream order preserves blocking)."""
    for fn in nc.m.functions:
        for blk in fn.blocks:
            newlist = []
            changed = False
            for ins in blk.instructions:
                si = ins.sync_info
                if si is not None and si.on_wait and len(si.on_wait) > limit:
                    waits = list(si.on_wait)
                    excess, keep = waits[:-limit], waits[-limit:]
                    for i, w in enumerate(excess):
                        nop = bass_rust.InstNoOp(
                            name=f"{ins.name}-wsplit{i}", ins=[], outs=[]
                        )
                        nop.engine = ins.engine
                        nop.sync_info = mybir.SyncInfo(on_wait=[w], on_update=[])
                        newlist.append(nop)
                    si.on_wait = keep
                    ins.sync_info = si
                    changed = True
                newlist.append(ins)
            if changed:
                blk.instructions = newlist


def _build():
    nc = bass.Bass()
    # A^T union band, 2 contraction double-chunks: [k, i, n] = AT[256c+128i+k]
    at0_d = nc.dram_tensor("at0", [128, 2, UNIW], F8, kind="ExternalInput")
    at1_d = nc.dram_tensor("at1", [128, 2, UNIW], F8, kind="ExternalInput")
    # -2*A rows (stationary): [k, i, m] over the core's 1024 rows
    lhs0_d = nc.dram_tensor("lhs0", [128, 2, ROWS], F8, kind="ExternalInput")
    lhs1_d = nc.dram_tensor("lhs1", [128, 2, ROWS], F8, kind="ExternalInput")
    sqn_d = nc.dram_tensor("sqn", [1, UNIW], F16, kind="ExternalInput")
    sqm_d = nc.dram_tensor("sqm", [128, 8], F32, kind="ExternalInput")
    # fold operands for band block 8 (union cols 4096..5120): row 0 carries
    # the rank-2 update [1; sq'_m] x [sq'_n; 1], rows 1-63 are zero padding
    # so the matmul runs at full partition width (tiny-K matmuls stall PE).
    w1_d = nc.dram_tensor("w1", [64, 2, ROWS], F8, kind="ExternalInput")
    i1_d = nc.dram_tensor("i1", [64, 2, 1024], F8, kind="ExternalInput")
    out_d = nc.dram_tensor("out", [ROWS, BANDW], F16, kind="ExternalOutput")

    with TileContext(nc) as tc:
        with (
            tc.tile_pool(name="const", bufs=1) as cpool,
            tc.tile_pool(name="ps", bufs=2, space="PSUM") as pspool,
            tc.tile_pool(name="t", bufs=5) as tpool,
            tc.tile_pool(name="u", bufs=5) as upool,
        ):
            # Allocate big tiles first: every slot below is a multiple of
            # 2048 B/partition, keeping all bases 64B-aligned (a 32B-aligned
            # DVE operand measured ~20% slower). 32B-slot tiles come last.
            atb = [cpool.tile([128, 2, UNIW], F8, tag=f"atb{ch}", name=f"atb{ch}")
                   for ch in range(2)]
            sqn = cpool.tile([128, UNIW], F16)
            lhs = [cpool.tile([128, 2, ROWS], F8, tag=f"lhs{ch}", name=f"lhs{ch}")
                   for ch in range(2)]
            w1 = cpool.tile([64, 2, ROWS], F8)
            i1 = cpool.tile([64, 2, 1024], F8)
            warm_in = cpool.tile([1, 128], F16)
            warm_act = cpool.tile([128, 16], F32)
            sqm = cpool.tile([128, 8], F32)
            b1024 = cpool.tile([128, 1], F32)

            nc.vector.memset(b1024[:], 2.0 * SQC)
            nc.vector.memset(warm_in[:], 1.0)
            nc.vector.memset(warm_act[:], 1.0)

            # PE clock-gate warmup (HAM ramp): short N=64 fp16 matmuls ramp
            # the clock without the 10 us a N=512 warmup costs, and bridge
            # the gap until the first A^T columns land.
            warm_ps = pspool.tile([128, 2048], F32, tag="ps")
            for _ in range(88):
                nc.tensor.matmul(
                    warm_ps[:, 0:64], warm_in[0:1, 0:128], warm_in[0:1, 0:64],
                    start=True, stop=True,
                )
            # ACT Sqrt table preload
            nc.scalar.activation(warm_act[:], warm_act[:], SQRT, bias=0.0)

            # Input DMAs, ordered by when compute needs them. atb slices move
            # both k-subtile rows per DMA (2 lines/partition).
            for ch, ld in enumerate((lhs0_d, lhs1_d)):
                nc.sync.dma_start(lhs[ch][:], ld[:])
            for ch, ad in enumerate((at0_d, at1_d)):
                nc.sync.dma_start(atb[ch][:, :, 0:2048], ad[:, :, 0:2048])
            nc.sync.dma_start(
                sqn[:, 0:2048], sqn_d[0:1, 0:2048].partition_broadcast(128)
            )
            nc.sync.dma_start(sqm[:], sqm_d[:])
            nc.sync.dma_start(
                sqn[:, 2048:UNIW],
                sqn_d[0:1, 2048:UNIW].partition_broadcast(128),
            )
            for lo, hi in ((2048, 4096), (4096, UNIW)):
                for ch, ad in enumerate((at0_d, at1_d)):
                    nc.sync.dma_start(atb[ch][:, :, lo:hi], ad[:, :, lo:hi])
            nc.sync.dma_start(w1[:], w1_d[:])
            nc.sync.dma_start(i1[:], i1_d[:])

            for s in range(2):      # strip = row half
                base = NB * s       # band offset in union cols
                ts_ = [tpool.tile([128, 4096], F16, tag="t", name=f"t{s}{m}")
                       for m in range(4)]
                us_ = [upool.tile([128, 4096], F16, tag="u", name=f"u{s}{m}")
                       for m in range(4)]
                # Strip 0 runs g-major (the g=0 wave only needs the first atb
                # columns, which land first); strip 1 runs m-major with its
                # fold first, so the trailing ACT+DMA chain stays short.
                if s == 0:
                    order = [(0, 0), (0, 1), (0, 2), (0, 3), "fold",
                             (1, 0), (1, 1), (1, 2), (1, 3)]
                else:
                    order = ["fold", (0, 0), (1, 0), (0, 1), (1, 1),
                             (0, 2), (1, 2), (0, 3), (1, 3)]
                for item in order:
                    if item == "fold":
                        # band block 8 for all four m-tiles of the strip
                        c0 = base + 4096
                        ps = pspool.tile([128, 2048], F32, tag="ps")
                        for m in range(4):
                            m0 = NB * s + 128 * m
                            for ch in range(2):
                                nc.tensor.matmul(
                                    ps[:, 512 * m:512 * (m + 1)],
                                    lhs[ch][:, 0:2, m0:m0 + 128],
                                    atb[ch][:, 0:2, c0:c0 + 512],
                                    start=(ch == 0), stop=False,
                                    perf_mode=DBL,
                                )
                            nc.tensor.matmul(
                                ps[:, 512 * m:512 * (m + 1)],
                                w1[:, 0:2, m0:m0 + 128],
                                i1[:, 0:2, 512 * s:512 * (s + 1)],
                                start=False, stop=True,
                                perf_mode=DBL,
                            )
                        uf = upool.tile([128, 4096], F16, tag="u")
                        nc.scalar.activation(
                            uf[:, 0:2048], ps[:], SQRT, bias=b1024[:, 0:1]
                        )
                        for m in range(4):
                            m0 = NB * s + 128 * m
                            nc.sync.dma_start(
                                out_d[m0:m0 + 128, 4096:4608],
                                uf[:, 512 * m:512 * (m + 1)],
                            )
                        continue
                    g, m = item
                    mt = 4 * s + m
                    m0 = NB * s + 128 * m
                    last = (s == 1 and m == 3)
                    c0 = base + 2048 * g
                    ps = pspool.tile([128, 2048], F32, tag="ps")
                    for b in range(4):
                        nb0 = c0 + 512 * b
                        for ch in range(2):
                            nc.tensor.matmul(
                                ps[:, 512 * b:512 * (b + 1)],
                                lhs[ch][:, 0:2, m0:m0 + 128],
                                atb[ch][:, 0:2, nb0:nb0 + 512],
                                start=(ch == 0), stop=(ch == 1),
                                perf_mode=DBL,
                            )
                    gs = slice(2048 * g, 2048 * (g + 1))
                    nc.vector.tensor_tensor(
                        ts_[m][:, gs], ps[:], sqn[:, c0:c0 + 2048], ADD
                    )
                    if last:  # shorter tail: per-group sqrt + store
                        nc.scalar.activation(
                            us_[m][:, gs], ts_[m][:, gs], SQRT,
                            bias=sqm[:, mt:mt + 1],
                        )
                        nc.sync.dma_start(
                            out_d[m0:m0 + 128, gs], us_[m][:, gs]
                        )
                    elif g == 1:  # one wide sqrt + store per m-tile
                        nc.scalar.activation(
                            us_[m][:], ts_[m][:], SQRT,
                            bias=sqm[:, mt:mt + 1],
                        )
                        nc.sync.dma_start(
                            out_d[m0:m0 + 128, 0:4096], us_[m][:]
                        )
    _split_excess_waits(nc)
    return nc


def prepare_in_maps(mapping: np.ndarray):
    mapping = np.ascontiguousarray(mapping, dtype=np.float32)
    assert mapping.shape == (N, D)
    a8 = mapping.astype(NP8)
    af = a8.astype(np.float32)
    # exact squared norms of the rounded points
    sq = np.einsum("nd,nd->n", af.astype(np.float64),
                   af.astype(np.float64)).astype(np.float64)
    lhs8 = (-2.0 * af).astype(NP8)           # exact: *2 shifts exponent
    at8 = np.ascontiguousarray(a8.T)         # [D, N]
    lhs8t = np.ascontiguousarray(lhs8.T)     # [D, N]
    sqc8 = np.clip(sq - SQC, -235.0, 235.0).astype(NP8)  # centered, fp8

    in_maps = []
    for c in range(NCORES):
        cols = (1024 * c + np.arange(UNIW)) % N
        atr = np.take(at8, cols, axis=1)     # [512, 5120]
        rows = slice(1024 * c, 1024 * c + ROWS)
        lhsr = lhs8t[:, rows]                # [512, 1024]

        def chunked(x, ch):
            # [256, W] rows 256ch..256ch+256 -> [128, 2, W]
            blk = x[256 * ch:256 * (ch + 1)]
            return np.ascontiguousarray(
                blk.reshape(2, 128, -1).transpose(1, 0, 2)
            )

        sqm = np.ascontiguousarray(
            sq[rows].reshape(8, 128).T.astype(np.float32)
        )                                    # [128, 8][p, mt]
        sqn = sq[cols].astype(np.float16).reshape(1, UNIW)
        # fold operands: row 0 = rank-2 update, rows 1-63 zero padding
        w1 = np.zeros((64, 2, ROWS), NP8)
        w1[0, 0, :] = NP8(1.0)
        w1[0, 1, :] = sqc8[rows]
        i1 = np.zeros((64, 2, 1024), NP8)
        i1[0, 0, :] = sqc8[cols[4096:5120]]
        i1[0, 1, :] = NP8(1.0)
        in_maps.append({
            "at0": chunked(atr, 0), "at1": chunked(atr, 1),
            "lhs0": chunked(lhsr, 0), "lhs1": chunked(lhsr, 1),
            "sqn": sqn, "sqm": sqm, "w1": w1, "i1": i1,
        })
    return in_maps


def assemble(results) -> np.ndarray:
    """Place the 16 computed band strips, mirror the missing blocks."""
    out = np.empty((N, N), dtype=np.float32)
    for c in range(NCORES):
        band = results[c]["out"].astype(np.float32)   # [1024, 4608]
        for s in range(2):
            r0 = 1024 * c + NB * s
            strip = band[NB * s:NB * s + NB]
            c0 = r0 % N
            w1 = min(BANDW, N - c0)
            out[r0:r0 + NB, c0:c0 + w1] = strip[:, :w1]
            if w1 < BANDW:
                out[r0:r0 + NB, 0:BANDW - w1] = strip[:, w1:]
    # mirror blocks with (C-R) mod 16 in 9..15 from their transposed partner
    for k in range(BAND, NBLK):
        for R in range(NBLK):
            C = (R + k) % NBLK
            out[R * NB:(R + 1) * NB, C * NB:(C + 1) * NB] = \
                out[C * NB:(C + 1) * NB, R * NB:(R + 1) * NB].T
    np.fill_diagonal(out, 0.0)
    return out


_NC_CACHE = {}


def kernel(mapping: np.ndarray) -> np.ndarray:
    in_maps = prepare_in_maps(mapping)
    if "nc" not in _NC_CACHE:
        _NC_CACHE["nc"] = _build()
    nc = _NC_CACHE["nc"]
    res = None
    for attempt in range(3):
        try:
            res = run_bass_kernel_spmd(nc, in_maps, core_ids=list(range(NCORES)))
            break
        except Exception:
            # transient device wedge; pause + retry
            if attempt == 2:
                raise
            import time
            time.sleep(20)
    return assemble([res.results[c] for c in range(NCORES)])


# revision 20
# speedup vs baseline: 1.2028x; 1.0041x over previous
"""Pairwise Euclidean distance matrix on 8 Trainium2 NeuronCores.

Problem: mapping [8192, 512] f32 -> out[i,j] = ||mapping_i - mapping_j||_2,
shape [8192, 8192] f32.

Strategy: symmetry-aware staircase sharding + fp8 DoubleRow matmuls.

The output is symmetric, so only ~half needs computing on device. Rows are
split into 16 blocks of 512; row-block R computes column blocks
C = R..R+8 (mod 16) - a 4608-wide rotated band. Every unordered block pair
{R, C} is covered (distance k=|C-R| mod 16 <= 8 directly, k > 8 via the
transposed partner), so the host mirrors the missing blocks. Core c owns
row-blocks {2c, 2c+1} (1024 rows, two 512-row strips); the two strips'
bands overlap so their union [1024c, 1024c+5120) mod 8192 is loaded once.
Work per core: 72 [128,512] output tiles = 56% of the dense row slab.

Math per tile: d^2 = sq_m + sq_n - 2 a_m.a_n on fp8(e4m3)-rounded points.
  - Gram: TensorE fp8 DoubleRow matmuls (2 contraction rows/partition,
    2x fp16 throughput; 512-dim contraction = 2 matmuls/tile). The moving
    operand is A^T (shared band); the stationary operand is -2*A rows.
  - Band blocks 0-7: DVE adds an fp16 sq_n broadcast row to PSUM
    ([128,2048] 4-bank ops), ACT computes sqrt(t + sq_m) with per-partition
    f32 bias ([128,2048] ops) -> fp16 out.
  - Band block 8: sq_n and sq_m ride into PSUM as one K=1-pair DoubleRow
    matmul (rank-2 update: 1*sq'_n + sq'_m*1, values centered by -512);
    ACT reads PSUM directly with constant bias 1024. This keeps DVE off
    ~1/9 of tiles, balancing DVE vs ACT vs PE (~35 us each).

Host side (cheap, O(N^2) only for unshard/mirror): fp8 rounding of points,
sq in f64, strip gather (mod-rotation), band placement, symmetric mirror of
the uncomputed blocks, diagonal zero. The device computes every distance
at least once.
"""

import numpy as np
import ml_dtypes
import bass_rust
import concourse.bass as bass
import concourse.mybir as mybir
from concourse.tile import TileContext
from concourse.bass_utils import run_bass_kernel_spmd


N = 8192
D = 512
NCORES = 8
NB = 512                  # block size (rows/cols)
NBLK = N // NB            # 16 row/col blocks
BAND = 9                  # col blocks computed per row block
BANDW = BAND * NB         # 4608
UNIW = BANDW + NB         # 5120: union of the two strips' bands
ROWS = 1024               # rows per core (2 strips)
F32 = mybir.dt.float32
F16 = mybir.dt.float16
F8 = mybir.dt.float8e4
NP8 = ml_dtypes.float8_e4m3
DBL = mybir.MatmulPerfMode.DoubleRow
ADD = mybir.AluOpType.add
SQRT = mybir.ActivationFunctionType.Sqrt
SQC = 512.0               # centering constant for fp8 sq values


def _split_excess_waits(nc, limit=1):
    """Walrus in this container rejects instructions with >1 sem-wait; hoist
    excess waits onto same-engine NoOps (stream order preserves blocking)."""
    for fn in nc.m.functions:
        for blk in fn.blocks:
            newlist = []
            changed = False
            for ins in blk.instructions:
                si = ins.sync_info
                if si is not None and si.on_wait and len(si.on_wait) > limit:
                    waits = list(si.on_wait)
                    excess, keep = waits[:-limit], waits[-limit:]
                    for i, w in enumerate(excess):
                        nop = bass_rust.InstNoOp(
                            name=f"{ins.name}-wsplit{i}", ins=[], outs=[]
                        )
                        nop.engine = ins.engine
                        nop.sync_info = mybir.SyncInfo(on_wait=[w], on_update=[])
                        newlist.append(nop)
                    si.on_wait = keep
                    ins.sync_info = si
                    changed = True
                newlist.append(ins)
            if changed:
                blk.instructions = newlist


def _build():
    nc = bass.Bass()
    # A^T union band, 2 contraction double-chunks: [k, i, n] = AT[256c+128i+k]
    at0_d = nc.dram_tensor("at0", [128, 2, UNIW], F8, kind="ExternalInput")
    at1_d = nc.dram_tensor("at1", [128, 2, UNIW], F8, kind="ExternalInput")
    # -2*A rows (stationary): [k, i, m] over the core's 1024 rows
    lhs0_d = nc.dram_tensor("lhs0", [128, 2, ROWS], F8, kind="ExternalInput")
    lhs1_d = nc.dram_tensor("lhs1", [128, 2, ROWS], F8, kind="ExternalInput")
    sqn_d = nc.dram_tensor("sqn", [1, UNIW], F16, kind="ExternalInput")
    sqm_d = nc.dram_tensor("sqm", [128, 8], F32, kind="ExternalInput")
    # fold operands for band block 8 (union cols 4096..5120): row 0 carries
    # the rank-2 update [1; sq'_m] x [sq'_n; 1], rows 1-63 are zero padding
    # so the matmul runs at full partition width (tiny-K matmuls stall PE).
    w1_d = nc.dram_tensor("w1", [64, 2, ROWS], F8, kind="ExternalInput")
    i1_d = nc.dram_tensor("i1", [64, 2, 1024], F8, kind="ExternalInput")
    out_d = nc.dram_tensor("out", [ROWS, BANDW], F16, kind="ExternalOutput")

    with TileContext(nc) as tc:
        with (
            tc.tile_pool(name="const", bufs=1) as cpool,
            tc.tile_pool(name="ps", bufs=2, space="PSUM") as pspool,
            tc.tile_pool(name="t", bufs=4) as tpool,
            tc.tile_pool(name="u", bufs=4) as upool,
        ):
            # Allocate big tiles first: every slot below is a multiple of
            # 2048 B/partition, keeping all bases 64B-aligned (a 32B-aligned
            # DVE operand measured ~20% slower). 32B-slot tiles come last.
            atb = [cpool.tile([128, 2, UNIW], F8, tag=f"atb{ch}", name=f"atb{ch}")
                   for ch in range(2)]
            sqn = cpool.tile([128, UNIW], F16)
            lhs = [cpool.tile([128, 2, ROWS], F8, tag=f"lhs{ch}", name=f"lhs{ch}")
                   for ch in range(2)]
            w1 = cpool.tile([64, 2, ROWS], F8)
            i1 = cpool.tile([64, 2, 1024], F8)
            warm_in = cpool.tile([1, 128], F16)
            warm_act = cpool.tile([128, 16], F32)
            sqm = cpool.tile([128, 8], F32)
            b1024 = cpool.tile([128, 1], F32)

            nc.vector.memset(b1024[:], 2.0 * SQC)
            nc.vector.memset(warm_in[:], 1.0)
            nc.vector.memset(warm_act[:], 1.0)

            # PE clock-gate warmup (HAM ramp): short N=64 fp16 matmuls ramp
            # the clock without the 10 us a N=512 warmup costs, and bridge
            # the gap until the first A^T columns land.
            warm_ps = pspool.tile([128, 2048], F32, tag="ps")
            for _ in range(88):
                nc.tensor.matmul(
                    warm_ps[:, 0:64], warm_in[0:1, 0:128], warm_in[0:1, 0:64],
                    start=True, stop=True,
                )
            # ACT Sqrt table preload
            nc.scalar.activation(warm_act[:], warm_act[:], SQRT, bias=0.0)

            # Input DMAs on the sync queue, ordered by when compute needs
            # them (outputs go on the gpsimd SWDGE queue to stay out of the
            # way). atb slices move both k-subtile rows per DMA.
            nc.sync.dma_start(sqm[:], sqm_d[:])
            for ch, ld in enumerate((lhs0_d, lhs1_d)):
                nc.sync.dma_start(lhs[ch][:], ld[:])
            for ch, ad in enumerate((at0_d, at1_d)):
                nc.sync.dma_start(atb[ch][:, :, 0:2048], ad[:, :, 0:2048])
            nc.sync.dma_start(
                sqn[:, 0:2048], sqn_d[0:1, 0:2048].partition_broadcast(128)
            )
            for ch, ad in enumerate((at0_d, at1_d)):
                nc.sync.dma_start(atb[ch][:, :, 2048:4096], ad[:, :, 2048:4096])
            nc.sync.dma_start(
                sqn[:, 2048:UNIW],
                sqn_d[0:1, 2048:UNIW].partition_broadcast(128),
            )
            for ch, ad in enumerate((at0_d, at1_d)):
                nc.sync.dma_start(atb[ch][:, :, 4096:UNIW], ad[:, :, 4096:UNIW])
            nc.sync.dma_start(w1[:], w1_d[:])
            nc.sync.dma_start(i1[:], i1_d[:])

            # Uniform per-group pipeline: mms -> DVE add -> ACT sqrt -> DMA,
            # every op <= one group wide so no engine builds a backlog.
            # Group order per strip keeps early groups on early atb columns;
            # folds sit mid-strip where the ACT queue is shallow.
            for s in range(2):      # strip = row half
                base = NB * s       # band offset in union cols
                if s == 0:
                    order = [(0, 0), (1, 0), (0, 1), (2, 0), (1, 1), (3, 0),
                             "fold", (2, 1), (3, 1)]
                else:
                    order = [(0, 0), (0, 1), (1, 0), (1, 1), "fold",
                             (2, 0), (2, 1), (3, 0), (3, 1)]
                for item in order:
                    if item == "fold":
                        # band block 8 for all four m-tiles of the strip
                        c0 = base + 4096
                        ps = pspool.tile([128, 2048], F32, tag="ps")
                        for m in range(4):
                            m0 = NB * s + 128 * m
                            for ch in range(2):
                                nc.tensor.matmul(
                                    ps[:, 512 * m:512 * (m + 1)],
                                    lhs[ch][:, 0:2, m0:m0 + 128],
                                    atb[ch][:, 0:2, c0:c0 + 512],
                                    start=(ch == 0), stop=False,
                                    perf_mode=DBL,
                                )
                            nc.tensor.matmul(
                                ps[:, 512 * m:512 * (m + 1)],
                                w1[:, 0:2, m0:m0 + 128],
                                i1[:, 0:2, 512 * s:512 * (s + 1)],
                                start=False, stop=True,
                                perf_mode=DBL,
                            )
                        uf = upool.tile([128, 2048], F16, tag="u")
                        nc.scalar.activation(
                            uf[:], ps[:], SQRT, bias=b1024[:, 0:1]
                        )
                        for m in range(4):
                            m0 = NB * s + 128 * m
                            nc.gpsimd.dma_start(
                                out_d[m0:m0 + 128, 4096:4608],
                                uf[:, 512 * m:512 * (m + 1)],
                            )
                        continue
                    m, g = item
                    mt = 4 * s + m
                    m0 = NB * s + 128 * m
                    c0 = base + 2048 * g
                    ps = pspool.tile([128, 2048], F32, tag="ps")
                    for b in range(4):
                        nb0 = c0 + 512 * b
                        for ch in range(2):
                            nc.tensor.matmul(
                                ps[:, 512 * b:512 * (b + 1)],
                                lhs[ch][:, 0:2, m0:m0 + 128],
                                atb[ch][:, 0:2, nb0:nb0 + 512],
                                start=(ch == 0), stop=(ch == 1),
                                perf_mode=DBL,
                            )
                    t = tpool.tile([128, 2048], F16, tag="t")
                    nc.vector.tensor_tensor(
                        t[:], ps[:], sqn[:, c0:c0 + 2048], ADD
                    )
                    u = upool.tile([128, 2048], F16, tag="u")
                    nc.scalar.activation(
                        u[:], t[:], SQRT, bias=sqm[:, mt:mt + 1]
                    )
                    nc.gpsimd.dma_start(
                        out_d[m0:m0 + 128, 2048 * g:2048 * (g + 1)], u[:]
                    )
    _split_excess_waits(nc)
    return nc


def prepare_in_maps(mapping: np.ndarray):
    mapping = np.ascontiguousarray(mapping, dtype=np.float32)
    assert mapping.shape == (N, D)
    a8 = mapping.astype(NP8)
    af = a8.astype(np.float32)
    # exact squared norms of the rounded points
    sq = np.einsum("nd,nd->n", af.astype(np.float64),
                   af.astype(np.float64)).astype(np.float64)
    lhs8 = (-2.0 * af).astype(NP8)           # exact: *2 shifts exponent
    at8 = np.ascontiguousarray(a8.T)         # [D, N]
    lhs8t = np.ascontiguousarray(lhs8.T)     # [D, N]
    sqc8 = np.clip(sq - SQC, -235.0, 235.0).astype(NP8)  # centered, fp8

    in_maps = []
    for c in range(NCORES):
        cols = (1024 * c + np.arange(UNIW)) % N
        atr = np.take(at8, cols, axis=1)     # [512, 5120]
        rows = slice(1024 * c, 1024 * c + ROWS)
        lhsr = lhs8t[:, rows]                # [512, 1024]

        def chunked(x, ch):
            # [256, W] rows 256ch..256ch+256 -> [128, 2, W]
            blk = x[256 * ch:256 * (ch + 1)]
            return np.ascontiguousarray(
                blk.reshape(2, 128, -1).transpose(1, 0, 2)
            )

        sqm = np.ascontiguousarray(
            sq[rows].reshape(8, 128).T.astype(np.float32)
        )                                    # [128, 8][p, mt]
        sqn = sq[cols].astype(np.float16).reshape(1, UNIW)
        # fold operands: row 0 = rank-2 update, rows 1-63 zero padding
        w1 = np.zeros((64, 2, ROWS), NP8)
        w1[0, 0, :] = NP8(1.0)
        w1[0, 1, :] = sqc8[rows]
        i1 = np.zeros((64, 2, 1024), NP8)
        i1[0, 0, :] = sqc8[cols[4096:5120]]
        i1[0, 1, :] = NP8(1.0)
        in_maps.append({
            "at0": chunked(atr, 0), "at1": chunked(atr, 1),
            "lhs0": chunked(lhsr, 0), "lhs1": chunked(lhsr, 1),
            "sqn": sqn, "sqm": sqm, "w1": w1, "i1": i1,
        })
    return in_maps


def assemble(results) -> np.ndarray:
    """Place the 16 computed band strips, mirror the missing blocks."""
    out = np.empty((N, N), dtype=np.float32)
    for c in range(NCORES):
        band = results[c]["out"].astype(np.float32)   # [1024, 4608]
        for s in range(2):
            r0 = 1024 * c + NB * s
            strip = band[NB * s:NB * s + NB]
            c0 = r0 % N
            w1 = min(BANDW, N - c0)
            out[r0:r0 + NB, c0:c0 + w1] = strip[:, :w1]
            if w1 < BANDW:
                out[r0:r0 + NB, 0:BANDW - w1] = strip[:, w1:]
    # mirror blocks with (C-R) mod 16 in 9..15 from their transposed partner
    for k in range(BAND, NBLK):
        for R in range(NBLK):
            C = (R + k) % NBLK
            out[R * NB:(R + 1) * NB, C * NB:(C + 1) * NB] = \
                out[C * NB:(C + 1) * NB, R * NB:(R + 1) * NB].T
    np.fill_diagonal(out, 0.0)
    return out


_NC_CACHE = {}


def kernel(mapping: np.ndarray) -> np.ndarray:
    in_maps = prepare_in_maps(mapping)
    if "nc" not in _NC_CACHE:
        _NC_CACHE["nc"] = _build()
    nc = _NC_CACHE["nc"]
    res = None
    for attempt in range(3):
        try:
            res = run_bass_kernel_spmd(nc, in_maps, core_ids=list(range(NCORES)))
            break
        except Exception:
            # transient device wedge; pause + retry
            if attempt == 2:
                raise
            import time
            time.sleep(20)
    return assemble([res.results[c] for c in range(NCORES)])


# revision 22
# speedup vs baseline: 1.2808x; 1.0649x over previous
"""Pairwise Euclidean distance matrix on 8 Trainium2 NeuronCores.

Problem: mapping [8192, 512] f32 -> out[i,j] = ||mapping_i - mapping_j||_2,
shape [8192, 8192] f32.

Strategy: symmetry-aware staircase sharding + fp8 DoubleRow matmuls.

The output is symmetric, so only ~half needs computing on device. Rows are
split into 16 blocks of 512; row-block R computes column blocks
C = R..R+8 (mod 16) - a 4608-wide rotated band. Every unordered block pair
{R, C} is covered (distance k=|C-R| mod 16 <= 8 directly, k > 8 via the
transposed partner), so the host mirrors the missing blocks. Core c owns
row-blocks {2c, 2c+1} (1024 rows, two 512-row strips); the two strips'
bands overlap so their union [1024c, 1024c+5120) mod 8192 is loaded once.
Work per core: 72 [128,512] output tiles = 56% of the dense row slab.

Math per tile: d^2 = sq_m + sq_n - 2 a_m.a_n on fp8(e4m3)-rounded points.
  - Gram: TensorE fp8 DoubleRow matmuls (2 contraction rows/partition,
    2x fp16 throughput; 512-dim contraction = 2 matmuls/tile). The moving
    operand is A^T (shared band); the stationary operand is -2*A rows.
  - Band blocks 0-7: DVE adds an fp16 sq_n broadcast row to PSUM
    ([128,2048] 4-bank ops), ACT computes sqrt(t + sq_m) with per-partition
    f32 bias ([128,2048] ops) -> fp16 out.
  - Band block 8: sq_n and sq_m ride into PSUM as one K=1-pair DoubleRow
    matmul (rank-2 update: 1*sq'_n + sq'_m*1, values centered by -512);
    ACT reads PSUM directly with constant bias 1024. This keeps DVE off
    ~1/9 of tiles, balancing DVE vs ACT vs PE (~35 us each).

Host side (cheap, O(N^2) only for unshard/mirror): fp8 rounding of points,
sq in f64, strip gather (mod-rotation), band placement, symmetric mirror of
the uncomputed blocks, diagonal zero. The device computes every distance
at least once.
"""

import numpy as np
import ml_dtypes
import bass_rust
import concourse.bass as bass
import concourse.mybir as mybir
from concourse.tile import TileContext
from concourse.bass_utils import run_bass_kernel_spmd


N = 8192
D = 512
NCORES = 8
NB = 512                  # block size (rows/cols)
NBLK = N // NB            # 16 row/col blocks
BAND = 9                  # col blocks computed per row block
BANDW = BAND * NB         # 4608
UNIW = BANDW + NB         # 5120: union of the two strips' bands
ROWS = 1024               # rows per core (2 strips)
F32 = mybir.dt.float32
F16 = mybir.dt.float16
F8 = mybir.dt.float8e4
NP8 = ml_dtypes.float8_e4m3
DBL = mybir.MatmulPerfMode.DoubleRow
ADD = mybir.AluOpType.add
SQRT = mybir.ActivationFunctionType.Sqrt
SQC = 512.0               # centering constant for fp8 sq values


def _split_excess_waits(nc, limit=1):
    """Walrus in this container rejects instructions with >1 sem-wait; hoist
    excess waits onto same-engine NoOps (stream order preserves blocking)."""
    for fn in nc.m.functions:
        for blk in fn.blocks:
            newlist = []
            changed = False
            for ins in blk.instructions:
                si = ins.sync_info
                if si is not None and si.on_wait and len(si.on_wait) > limit:
                    waits = list(si.on_wait)
                    excess, keep = waits[:-limit], waits[-limit:]
                    for i, w in enumerate(excess):
                        nop = bass_rust.InstNoOp(
                            name=f"{ins.name}-wsplit{i}", ins=[], outs=[]
                        )
                        nop.engine = ins.engine
                        nop.sync_info = mybir.SyncInfo(on_wait=[w], on_update=[])
                        newlist.append(nop)
                    si.on_wait = keep
                    ins.sync_info = si
                    changed = True
                newlist.append(ins)
            if changed:
                blk.instructions = newlist


def _build():
    nc = bass.Bass()
    # A^T union band, 2 contraction double-chunks: [k, i, n] = AT[256c+128i+k]
    at0_d = nc.dram_tensor("at0", [128, 2, UNIW], F8, kind="ExternalInput")
    at1_d = nc.dram_tensor("at1", [128, 2, UNIW], F8, kind="ExternalInput")
    # -2*A rows (stationary): [k, i, m] over the core's 1024 rows
    lhs0_d = nc.dram_tensor("lhs0", [128, 2, ROWS], F8, kind="ExternalInput")
    lhs1_d = nc.dram_tensor("lhs1", [128, 2, ROWS], F8, kind="ExternalInput")
    sqn_d = nc.dram_tensor("sqn", [1, UNIW], F16, kind="ExternalInput")
    sqm_d = nc.dram_tensor("sqm", [128, 8], F32, kind="ExternalInput")
    # fold operands for band block 8 (union cols 4096..5120): row 0 carries
    # the rank-2 update [1; sq'_m] x [sq'_n; 1], rows 1-63 are zero padding
    # so the matmul runs at full partition width (tiny-K matmuls stall PE).
    w1_d = nc.dram_tensor("w1", [64, 2, ROWS], F8, kind="ExternalInput")
    i1_d = nc.dram_tensor("i1", [64, 2, 1024], F8, kind="ExternalInput")
    out_d = nc.dram_tensor("out", [ROWS, BANDW], F16, kind="ExternalOutput")

    with TileContext(nc) as tc:
        with (
            tc.tile_pool(name="const", bufs=1) as cpool,
            tc.tile_pool(name="ps", bufs=2, space="PSUM") as pspool,
            tc.tile_pool(name="t", bufs=4) as tpool,
            tc.tile_pool(name="u", bufs=4) as upool,
        ):
            # Allocate big tiles first: every slot below is a multiple of
            # 2048 B/partition, keeping all bases 64B-aligned (a 32B-aligned
            # DVE operand measured ~20% slower). 32B-slot tiles come last.
            atb = [cpool.tile([128, 2, UNIW], F8, tag=f"atb{ch}", name=f"atb{ch}")
                   for ch in range(2)]
            sqn = cpool.tile([128, UNIW], F16)
            lhs = [cpool.tile([128, 2, ROWS], F8, tag=f"lhs{ch}", name=f"lhs{ch}")
                   for ch in range(2)]
            w1 = cpool.tile([64, 2, ROWS], F8)
            i1 = cpool.tile([64, 2, 1024], F8)
            warm_in = cpool.tile([1, 128], F16)
            warm_act = cpool.tile([128, 16], F32)
            sqm = cpool.tile([128, 8], F32)
            b1024 = cpool.tile([128, 1], F32)

            nc.vector.memset(b1024[:], 2.0 * SQC)
            nc.vector.memset(warm_in[:], 1.0)
            nc.vector.memset(warm_act[:], 1.0)

            # PE clock-gate warmup (HAM ramp): short N=64 fp16 matmuls ramp
            # the clock without the 10 us a N=512 warmup costs, and bridge
            # the gap until the first A^T columns land.
            warm_ps = pspool.tile([128, 2048], F32, tag="ps")
            for _ in range(120):
                nc.tensor.matmul(
                    warm_ps[:, 0:64], warm_in[0:1, 0:128], warm_in[0:1, 0:64],
                    start=True, stop=True,
                )
            # ACT Sqrt table preload
            nc.scalar.activation(warm_act[:], warm_act[:], SQRT, bias=0.0)

            # Input DMAs on the sync queue, ordered by when compute needs
            # them (outputs go on the gpsimd SWDGE queue to stay out of the
            # way). atb slices move both k-subtile rows per DMA.
            nc.sync.dma_start(sqm[:], sqm_d[:])
            for ch, ld in enumerate((lhs0_d, lhs1_d)):
                nc.sync.dma_start(lhs[ch][:], ld[:])
            for ch, ad in enumerate((at0_d, at1_d)):
                nc.sync.dma_start(atb[ch][:, :, 0:2048], ad[:, :, 0:2048])
            nc.sync.dma_start(
                sqn[:, 0:2048], sqn_d[0:1, 0:2048].partition_broadcast(128)
            )
            for ch, ad in enumerate((at0_d, at1_d)):
                nc.sync.dma_start(atb[ch][:, :, 2048:4096], ad[:, :, 2048:4096])
            nc.sync.dma_start(
                sqn[:, 2048:UNIW],
                sqn_d[0:1, 2048:UNIW].partition_broadcast(128),
            )
            for ch, ad in enumerate((at0_d, at1_d)):
                nc.sync.dma_start(atb[ch][:, :, 4096:UNIW], ad[:, :, 4096:UNIW])
            nc.sync.dma_start(w1[:], w1_d[:])
            nc.sync.dma_start(i1[:], i1_d[:])

            # Uniform per-group pipeline: mms -> DVE add -> ACT sqrt -> DMA,
            # every op <= one group wide so no engine builds a backlog.
            # Group order per strip keeps early groups on early atb columns;
            # folds sit mid-strip where the ACT queue is shallow.
            for s in range(2):      # strip = row half
                base = NB * s       # band offset in union cols
                if s == 0:
                    order = [(0, 0), (1, 0), (0, 1), (2, 0), (1, 1), (3, 0),
                             (2, 1), (3, 1), "fold"]
                else:
                    order = [(0, 0), (0, 1), (1, 0), (1, 1), (2, 0),
                             (2, 1), (3, 0), (3, 1), "fold"]
                for item in order:
                    if item == "fold":
                        # band block 8 for all four m-tiles of the strip
                        c0 = base + 4096
                        ps = pspool.tile([128, 2048], F32, tag="ps")
                        for m in range(4):
                            m0 = NB * s + 128 * m
                            for ch in range(2):
                                nc.tensor.matmul(
                                    ps[:, 512 * m:512 * (m + 1)],
                                    lhs[ch][:, 0:2, m0:m0 + 128],
                                    atb[ch][:, 0:2, c0:c0 + 512],
                                    start=(ch == 0), stop=False,
                                    perf_mode=DBL,
                                )
                            nc.tensor.matmul(
                                ps[:, 512 * m:512 * (m + 1)],
                                w1[:, 0:2, m0:m0 + 128],
                                i1[:, 0:2, 512 * s:512 * (s + 1)],
                                start=False, stop=True,
                                perf_mode=DBL,
                            )
                        uf = upool.tile([128, 2048], F16, tag="u")
                        nc.scalar.activation(
                            uf[:], ps[:], SQRT, bias=b1024[:, 0:1]
                        )
                        for m in range(4):
                            m0 = NB * s + 128 * m
                            nc.gpsimd.dma_start(
                                out_d[m0:m0 + 128, 4096:4608],
                                uf[:, 512 * m:512 * (m + 1)],
                            )
                        continue
                    m, g = item
                    mt = 4 * s + m
                    m0 = NB * s + 128 * m
                    c0 = base + 2048 * g
                    ps = pspool.tile([128, 2048], F32, tag="ps")
                    for b in range(4):
                        nb0 = c0 + 512 * b
                        for ch in range(2):
                            nc.tensor.matmul(
                                ps[:, 512 * b:512 * (b + 1)],
                                lhs[ch][:, 0:2, m0:m0 + 128],
                                atb[ch][:, 0:2, nb0:nb0 + 512],
                                start=(ch == 0), stop=(ch == 1),
                                perf_mode=DBL,
                            )
                    t = tpool.tile([128, 2048], F16, tag="t")
                    nc.vector.tensor_tensor(
                        t[:], ps[:], sqn[:, c0:c0 + 2048], ADD
                    )
                    u = upool.tile([128, 2048], F16, tag="u")
                    nc.scalar.activation(
                        u[:], t[:], SQRT, bias=sqm[:, mt:mt + 1]
                    )
                    nc.gpsimd.dma_start(
                        out_d[m0:m0 + 128, 2048 * g:2048 * (g + 1)], u[:]
                    )
    _split_excess_waits(nc)
    return nc


def prepare_in_maps(mapping: np.ndarray):
    mapping = np.ascontiguousarray(mapping, dtype=np.float32)
    assert mapping.shape == (N, D)
    a8 = mapping.astype(NP8)
    af = a8.astype(np.float32)
    # exact squared norms of the rounded points
    sq = np.einsum("nd,nd->n", af.astype(np.float64),
                   af.astype(np.float64)).astype(np.float64)
    lhs8 = (-2.0 * af).astype(NP8)           # exact: *2 shifts exponent
    at8 = np.ascontiguousarray(a8.T)         # [D, N]
    lhs8t = np.ascontiguousarray(lhs8.T)     # [D, N]
    sqc8 = np.clip(sq - SQC, -235.0, 235.0).astype(NP8)  # centered, fp8

    in_maps = []
    for c in range(NCORES):
        cols = (1024 * c + np.arange(UNIW)) % N
        atr = np.take(at8, cols, axis=1)     # [512, 5120]
        rows = slice(1024 * c, 1024 * c + ROWS)
        lhsr = lhs8t[:, rows]                # [512, 1024]

        def chunked(x, ch):
            # [256, W] rows 256ch..256ch+256 -> [128, 2, W]
            blk = x[256 * ch:256 * (ch + 1)]
            return np.ascontiguousarray(
                blk.reshape(2, 128, -1).transpose(1, 0, 2)
            )

        sqm = np.ascontiguousarray(
            sq[rows].reshape(8, 128).T.astype(np.float32)
        )                                    # [128, 8][p, mt]
        sqn = sq[cols].astype(np.float16).reshape(1, UNIW)
        # fold operands: row 0 = rank-2 update, rows 1-63 zero padding
        w1 = np.zeros((64, 2, ROWS), NP8)
        w1[0, 0, :] = NP8(1.0)
        w1[0, 1, :] = sqc8[rows]
        i1 = np.zeros((64, 2, 1024), NP8)
        i1[0, 0, :] = sqc8[cols[4096:5120]]
        i1[0, 1, :] = NP8(1.0)
        in_maps.append({
            "at0": chunked(atr, 0), "at1": chunked(atr, 1),
            "lhs0": chunked(lhsr, 0), "lhs1": chunked(lhsr, 1),
            "sqn": sqn, "sqm": sqm, "w1": w1, "i1": i1,
        })
    return in_maps


def assemble(results) -> np.ndarray:
    """Place the 16 computed band strips, mirror the missing blocks."""
    out = np.empty((N, N), dtype=np.float32)
    for c in range(NCORES):
        band = results[c]["out"].astype(np.float32)   # [1024, 4608]
        for s in range(2):
            r0 = 1024 * c + NB * s
            strip = band[NB * s:NB * s + NB]
            c0 = r0 % N
            w1 = min(BANDW, N - c0)
            out[r0:r0 + NB, c0:c0 + w1] = strip[:, :w1]
            if w1 < BANDW:
                out[r0:r0 + NB, 0:BANDW - w1] = strip[:, w1:]
    # mirror blocks with (C-R) mod 16 in 9..15 from their transposed partner
    for k in range(BAND, NBLK):
        for R in range(NBLK):
            C = (R + k) % NBLK
            out[R * NB:(R + 1) * NB, C * NB:(C + 1) * NB] = \
                out[C * NB:(C + 1) * NB, R * NB:(R + 1) * NB].T
    np.fill_diagonal(out, 0.0)
    return out


_NC_CACHE = {}


def kernel(mapping: np.ndarray) -> np.ndarray:
    in_maps = prepare_in_maps(mapping)
    if "nc" not in _NC_CACHE:
        _NC_CACHE["nc"] = _build()
    nc = _NC_CACHE["nc"]
    res = None
    for attempt in range(3):
        try:
            res = run_bass_kernel_spmd(nc, in_maps, core_ids=list(range(NCORES)))
            break
        except Exception:
            # transient device wedge; pause + retry
            if attempt == 2:
                raise
            import time
            time.sleep(20)
    return assemble([res.results[c] for c in range(NCORES)])
